# revision 1
# baseline (speedup 1.0000x reference)
"""Trainium2 Bass kernel for ContinuousIntegratedKoopmanOperator.

reference: odeint(dz/dt = z @ W) sampled at t = DT*[1..T], y0 = x at t[0].
Closed form (time-invariant linear ODE): out[:, j, :] = x @ expm(DT*j*W).

Strategy:
  host: compute Mj = expm(DT*j*W) for j=0..T-1 in float64, cast to f32,
        concat to M (128, T*128).
  device (8 cores, batch-sharded 1024 rows each):
        out_tile[r, j*128+d] = sum_k x[r,k] * M[k, j*128+d]
        i.e. 8 batch tiles x 16 j-blocks of (128x128)@(128x512) fp32 matmuls,
        PSUM bank rotation, DVE drain to staging, 4MB contiguous DMA out.
"""
import numpy as np

DT = 0.01
B, D, T = 8192, 128, 64
NCORES = 8
BSH = B // NCORES          # 1024 rows per core
NTILES = BSH // 128        # 8 batch tiles per core
BW = 512                   # j-block width (4 j's of 128)
NBLK = (T * D) // BW       # 16 blocks
NPSUM = 8                  # psum banks rotated

_CACHE = {}


def _expm_table(W: np.ndarray) -> np.ndarray:
    """M (D, T*D) float32: columns [j*D:(j+1)*D] = expm(DT*j*W), computed in f64."""
    A = DT * W.astype(np.float64)
    # Taylor series; ||A|| ~ 0.02 so ~20 terms reach f64 machine eps.
    M1 = np.eye(D, dtype=np.float64)
    term = np.eye(D, dtype=np.float64)
    for n in range(1, 24):
        term = term @ A / n
        M1 += term
    Ms = np.empty((T, D, D), dtype=np.float64)
    Ms[0] = np.eye(D)
    for j in range(1, T):
        Ms[j] = Ms[j - 1] @ M1
    return np.ascontiguousarray(Ms.transpose(1, 0, 2).reshape(D, T * D)).astype(np.float32)


def _build_nc():
    import concourse.bass as bass
    import concourse.mybir as mybir

    f32 = mybir.dt.float32
    nc = bass.Bass(trn_type="TRN2")
    xT_d = nc.dram_tensor("xT", (D, BSH), f32, kind="ExternalInput")
    M_d = nc.dram_tensor("M", (D, T * D), f32, kind="ExternalInput")
    out_d = nc.dram_tensor("out", (BSH, T * D), f32, kind="ExternalOutput")

    xT_s = nc.alloc_sbuf_tensor("xT_s", [D, BSH], f32)
    M_s = nc.alloc_sbuf_tensor("M_s", [D, T * D], f32)
    stg = [nc.alloc_sbuf_tensor(f"stg{p}", [128, NBLK * BW], f32) for p in range(2)]
    psum = nc.alloc_psum_tensor("acc", [128, NPSUM * 512], f32)

    with (
        nc.Block() as block,
        nc.semaphore("s_load") as s_load,
        nc.semaphore("s_mm") as s_mm,
        nc.semaphore("s_drain") as s_drain,
        nc.semaphore("s_out") as s_out,
    ):
        @block.sync
        def _(sync):
            sync.dma_start(out=xT_s[:], in_=xT_d[:]).then_inc(s_load, 16)
            for b in range(NBLK):
                sync.dma_start(out=M_s[:, b * BW:(b + 1) * BW],
                               in_=M_d[:, b * BW:(b + 1) * BW]).then_inc(s_load, 16)
            for i in range(NTILES):
                sync.wait_ge(s_drain, NBLK * (i + 1))
                sync.dma_start(out=out_d[i * 128:(i + 1) * 128, :],
                               in_=stg[i % 2][:]).then_inc(s_out, 16)

        @block.tensor
        def _(tensor):
            for i in range(NTILES):
                for b in range(NBLK):
                    k = i * NBLK + b
                    if i == 0:
                        tensor.wait_ge(s_load, 16 * (b + 2))
                    if k >= NPSUM:
                        tensor.wait_ge(s_drain, k - NPSUM + 1)
                    pb = (k % NPSUM) * 512
                    tensor.matmul(psum[:, pb:pb + 512],
                                  xT_s[:, i * 128:(i + 1) * 128],
                                  M_s[:, b * BW:(b + 1) * BW],
                                  start=True, stop=True).then_inc(s_mm, 1)

        @block.vector
        def _(vector):
            for i in range(NTILES):
                for b in range(NBLK):
                    k = i * NBLK + b
                    if b == 0 and i >= 2:
                        vector.wait_ge(s_out, 16 * (i - 1))
                    vector.wait_ge(s_mm, k + 1)
                    pb = (k % NPSUM) * 512
                    vector.tensor_copy(out=stg[i % 2][:, b * BW:(b + 1) * BW],
                                       in_=psum[:, pb:pb + 512]).then_inc(s_drain, 1)

    return nc


def run_on_device(x: np.ndarray, Mcat: np.ndarray, trace: bool = False):
    """Shard x across 8 cores, run, return (out (B,T,D), results_obj)."""
    from concourse.bass_utils import run_bass_kernel_spmd

    if "nc" not in _CACHE:
        _CACHE["nc"] = _build_nc()
    nc = _CACHE["nc"]

    in_maps = []
    for c in range(NCORES):
        xT_c = np.ascontiguousarray(x[c * BSH:(c + 1) * BSH].T)
        in_maps.append({"xT": xT_c, "M": Mcat})

    res = run_bass_kernel_spmd(nc, in_maps, core_ids=list(range(NCORES)), trace=trace)
    out = np.empty((B, T, D), dtype=np.float32)
    for c in range(NCORES):
        out[c * BSH:(c + 1) * BSH] = res.results[c]["out"].reshape(BSH, T, D)
    return out, res


def kernel(x, W, T):
    x = np.asarray(x, dtype=np.float32)
    W = np.asarray(W, dtype=np.float32)
    assert int(T) == 64 and x.shape == (B, D) and W.shape == (D, D)
    Mcat = _expm_table(W)
    out, _ = run_on_device(x, Mcat, trace=False)
    return out
